# revision 2
# baseline (speedup 1.0000x reference)
"""GAT (graph attention) Bass kernel for Trainium2, 8-core SPMD — v2.

Strategy: receiver-per-partition windows + chunked indirect-DMA gathers.

Host sorts active nodes by degree and packs them 128 per window (one SBUF
partition per receiver). Windows are dealt round-robin to the 8 cores so
every core runs one shared instruction stream; the per-window slot count
K[w] (edge blocks of 128 slots) is the max over the 8 cores' windows.

Device kernel, per core:
  phase A: tab[n] = [h(64) | s1(4) | s2(4) | pad] fp16 256B rows, written
           block-permuted so the stores are fully contiguous (the gather
           indices absorb the permutation); one sentinel row at npad with
           h=0, s1=-100 (=> pad slots exp to exactly 0 in fp16).
  phase B: per chunk of windows, ONE indirect DMA ([128, nb] i32 offsets)
           fetches all sender rows and one more fetches the receiver rows.
           Compute is pure DVE/ACT per partition: logit = s1 + s2(recv),
           leaky-relu, exp(.-3.5) (softmax-shift-invariant), then weighted
           free-axis reductions. No matmuls in phase B, no collectives.

Host scatters the staged [128, 64] window outputs back to node order.
"""

import os
import sys

import numpy as np

for _p in ("/opt/trn_rl_repo", os.path.expanduser("~/.axon_site/_ro/trn_rl_repo")):
    if os.path.isdir(_p) and _p not in sys.path:
        sys.path.insert(0, _p)

P = 128
XTILE = 1024                 # phase-A node super-tile
NBLK = XTILE // P            # 8
TCOLS = 128                  # fp16 table row = 256B
HEADS = 4
UNITS = 16
HU = HEADS * UNITS           # 64
S1OFF = HU                   # cols 64:68 = s1
S2OFF = HU + HEADS           # cols 68:72 = s2
WC = HU + 2 * HEADS          # 72 written cols
LEAKY_ALPHA = 0.2
CSHIFT = 3.5                 # global exp shift (softmax-invariant)
S1_SENTINEL = -100.0         # sentinel row: exp(leaky(s1+s2)-c) == 0 in fp16
BCAP = 128                   # max edge blocks per gather chunk
WCAP = 8                     # max windows per gather chunk
GCALL = 1                    # blocks per indirect DMA call (multi-col offset
                             # APs gather garbage on HW; keep 1)
ABLATE = "full"              # dev-only: "phaseA" | "nocompute"
REPS = 1                     # dev-only: replicate kernel body for timing


def _perm(n):
    """Node id -> permuted table row (phase-A stores become contiguous)."""
    n = np.asarray(n)
    t, r = n // XTILE, n % XTILE
    return t * XTILE + (r % P) * NBLK + (r // P)


def _build_host_data(x, edge_index, W, att_w1, att_w2, n_cores):
    n_nodes, in_feat = x.shape
    snd = edge_index[:, 0].astype(np.int64)
    rcv = edge_index[:, 1].astype(np.int64)

    ntiles = -(-n_nodes // XTILE)
    npad = ntiles * XTILE
    sent = npad  # sentinel row index

    deg = np.bincount(rcv, minlength=n_nodes)
    active = np.nonzero(deg > 0)[0]
    order_n = active[np.argsort(deg[active], kind="stable")]

    wtot = -(-len(order_n) // P)
    nw = -(-wtot // n_cores)
    wpad = nw * n_cores
    win_nodes_g = np.full((wpad, P), -1, dtype=np.int64)
    win_nodes_g.reshape(-1)[: len(order_n)] = order_n

    deg_g = np.where(win_nodes_g >= 0, deg[win_nodes_g], 0)
    k_g = deg_g.max(axis=1)
    # per-local-window block cap: max over the n_cores interleaved windows
    K = k_g.reshape(nw, n_cores).max(axis=1).astype(np.int64)

    # chunking: greedy, <= BCAP blocks and <= WCAP windows per chunk
    chunks = []  # list of (w0, nwin)
    w = 0
    while w < nw:
        w0 = w
        blocks = 0
        while w < nw and (w - w0) < WCAP and (blocks + K[w]) <= max(BCAP, K[w]):
            blocks += K[w]
            w += 1
        chunks.append((w0, w - w0))

    # node -> (core, local w, partition)
    node_c = np.full(n_nodes, -1, dtype=np.int64)
    node_w = np.zeros(n_nodes, dtype=np.int64)
    node_p = np.zeros(n_nodes, dtype=np.int64)
    gwin = np.repeat(np.arange(wpad), P).reshape(wpad, P)
    valid = win_nodes_g >= 0
    vn = win_nodes_g[valid]
    node_c[vn] = gwin[valid] % n_cores
    node_w[vn] = gwin[valid] // n_cores
    node_p[vn] = np.tile(np.arange(P), wpad).reshape(wpad, P)[valid]

    # edge -> slot k within its receiver's run
    eorder = np.argsort(rcv, kind="stable")
    rs = rcv[eorder]
    ss = snd[eorder]
    starts = np.zeros(n_nodes + 1, dtype=np.int64)
    starts[1:] = np.cumsum(deg)
    k_e = np.arange(len(rs)) - starts[rs]
    perm_ss = _perm(ss)

    base = np.zeros(nw + 1, dtype=np.int64)
    base[1:] = np.cumsum(K)
    btot = int(base[-1])  # total sender blocks per core

    xT16 = np.zeros((in_feat, npad), dtype=np.float16)
    xT16[:, :n_nodes] = np.ascontiguousarray(x.T).astype(np.float16)

    # wcat = [W | W@A1 | W@A2] fp16  [in_feat, 72]
    A12 = np.zeros((HU, 2 * HEADS), dtype=np.float32)
    for h in range(HEADS):
        A12[h * UNITS:(h + 1) * UNITS, h] = att_w1[h, 0]
        A12[h * UNITS:(h + 1) * UNITS, HEADS + h] = att_w2[h, 0]
    wcat = np.zeros((in_feat, WC), dtype=np.float32)
    wcat[:, :HU] = W
    wcat[:, HU:] = W @ A12
    wcat16 = wcat.astype(np.float16)

    zrow = np.zeros((1, TCOLS), dtype=np.float16)
    zrow[0, S1OFF:S1OFF + HEADS] = S1_SENTINEL

    per_core = []
    win_nodes_c_all = []
    for c in range(n_cores):
        wn = win_nodes_g[c::n_cores]  # [nw, 128]
        emask = node_c[rs] == c
        er = rs[emask]
        ew = node_w[er]
        ep = node_p[er]
        ek = k_e[emask]

        sidx = np.full((btot, P), sent, dtype=np.int32)  # [block, partition]
        sidx[base[ew] + ek, ep] = perm_ss[emask].astype(np.int32)
        ridx = np.where(wn >= 0, _perm(np.maximum(wn, 0)), sent).astype(np.int32)

        per_core.append({
            "xT16": xT16,
            "wcat": wcat16,
            "zrow": zrow,
            "sidx": np.ascontiguousarray(sidx.T),   # [128, btot] i32
            "ridx": np.ascontiguousarray(ridx.T),   # [128, nw] i32
        })
        win_nodes_c_all.append(wn)

    plan = {
        "npad": npad, "ntiles": ntiles, "nw": nw,
        "K": K.tolist(), "base": base.tolist(), "btot": btot,
        "chunks": chunks, "in_feat": in_feat,
    }
    host = {"plan": plan, "win_nodes": win_nodes_c_all, "n_nodes": n_nodes}
    return host, per_core


def _build_bass(plan):
    from concourse import bacc, mybir, tile
    import concourse.bass as bass

    f16 = mybir.dt.float16
    f32 = mybir.dt.float32
    i32 = mybir.dt.int32

    npad = plan["npad"]
    ntiles = plan["ntiles"]
    nw = plan["nw"]
    K = plan["K"]
    base = plan["base"]
    btot = plan["btot"]
    chunks = plan["chunks"]
    in_feat = plan["in_feat"]

    nc = bacc.Bacc("TRN2", target_bir_lowering=False, debug=False,
                   enable_asserts=False, num_devices=1)

    xT_d = nc.dram_tensor("xT16", [in_feat, npad], f16, kind="ExternalInput").ap()
    wcat_d = nc.dram_tensor("wcat", [in_feat, WC], f16, kind="ExternalInput").ap()
    zrow_d = nc.dram_tensor("zrow", [1, TCOLS], f16, kind="ExternalInput").ap()
    sidx_d = nc.dram_tensor("sidx", [P, btot], i32, kind="ExternalInput").ap()
    ridx_d = nc.dram_tensor("ridx", [P, nw], i32, kind="ExternalInput").ap()

    out_d = nc.dram_tensor("staged", [nw * P, HU], f32, kind="ExternalOutput").ap()
    tab_d = nc.dram_tensor("tab", [npad + 1, TCOLS], f16, kind="Internal").ap()

    with tile.TileContext(nc) as tc:
        with tc.tile_pool(name="consts", bufs=1) as cpool:
            wcat_sb = cpool.tile([in_feat, WC], f16, tag="wcat")
            nc.sync.dma_start(out=wcat_sb[:], in_=wcat_d[:])
            sidx_sb = cpool.tile([P, btot], i32, tag="sidx")
            nc.sync.dma_start(out=sidx_sb[:], in_=sidx_d[:])
            ridx_sb = cpool.tile([P, nw], i32, tag="ridx")
            nc.sync.dma_start(out=ridx_sb[:], in_=ridx_d[:])
            zr_sb = cpool.tile([1, TCOLS], f16, tag="zrow")
            nc.sync.dma_start(out=zr_sb[:], in_=zrow_d[:])
            nc.sync.dma_start(out=tab_d[npad:npad + 1, :], in_=zr_sb[:])
            cbias = cpool.tile([P, 1], f32, tag="cbias")
            nc.gpsimd.memset(cbias[:], -CSHIFT)
            zpad = cpool.tile([P, NBLK * (TCOLS - WC)], f16, tag="zpad")
            nc.gpsimd.memset(zpad[:], 0.0)

            # ---- phase A: node table ----
            with tc.tile_pool(name="pa_x", bufs=3) as pax, \
                 tc.tile_pool(name="pa_ps", bufs=3, space="PSUM") as paps, \
                 tc.tile_pool(name="pa_hs", bufs=3) as pahs:
              for _rep in range(REPS):
                for t in range(ntiles):
                    xt = pax.tile([in_feat, XTILE], f16, tag="xt")
                    nc.sync.dma_start(
                        out=xt[:], in_=xT_d[:, t * XTILE:(t + 1) * XTILE])
                    hst = pahs.tile([P, NBLK * TCOLS], f16, tag="hst")
                    hst3 = hst[:].rearrange("p (i c) -> p i c", c=TCOLS)
                    nc.vector.tensor_copy(
                        out=hst3[:, :, WC:TCOLS],
                        in_=zpad[:].rearrange("p (i c) -> p i c",
                                              c=TCOLS - WC))
                    half = NBLK // 2
                    for g in range(2):
                        ps = paps.tile([P, half * WC], f32, tag="ps")
                        for i in range(half):
                            b = g * half + i
                            nc.tensor.matmul(
                                out=ps[:, i * WC:(i + 1) * WC],
                                lhsT=xt[:, b * P:(b + 1) * P],
                                rhs=wcat_sb[:], start=True, stop=True)
                        nc.vector.tensor_copy(
                            out=hst3[:, g * half:(g + 1) * half, 0:WC],
                            in_=ps[:].rearrange("p (i c) -> p i c", c=WC))
                    nc.sync.dma_start(
                        out=tab_d[t * XTILE:(t + 1) * XTILE, :].rearrange(
                            "(p i) c -> p i c", p=P),
                        in_=hst3)

            # ---- phase B: windows ----
            if ABLATE != "phaseA":
              with tc.tile_pool(name="pb_hs", bufs=2) as pbh, \
                   tc.tile_pool(name="pb_rg", bufs=2) as pbr, \
                   tc.tile_pool(name="pb_w", bufs=3) as pbw, \
                   tc.tile_pool(name="pb_o", bufs=2) as pbo:
                for _rep in range(REPS):
                  for (w0, nwin) in chunks:
                    nb = sum(K[w0:w0 + nwin])
                    if nb == 0:
                        continue
                    hs = pbh.tile([P, nb * TCOLS], f16, tag="hs")
                    hs3 = hs[:].rearrange("p (j c) -> p j c", c=TCOLS)
                    for b0 in range(0, nb, GCALL):
                        b1 = min(b0 + GCALL, nb)
                        nc.gpsimd.indirect_dma_start(
                            out=hs3[:, b0:b1, :] if b1 - b0 > 1
                            else hs3[:, b0, :],
                            out_offset=None, in_=tab_d[:],
                            in_offset=bass.IndirectOffsetOnAxis(
                                ap=sidx_sb[:, base[w0] + b0:base[w0] + b1],
                                axis=0))
                    rg = pbr.tile([P, nwin * TCOLS], f16, tag="rg")
                    rg3 = rg[:].rearrange("p (j c) -> p j c", c=TCOLS)
                    for b0 in range(0, nwin, GCALL):
                        b1 = min(b0 + GCALL, nwin)
                        nc.gpsimd.indirect_dma_start(
                            out=rg3[:, b0:b1, :] if b1 - b0 > 1
                            else rg3[:, b0, :],
                            out_offset=None, in_=tab_d[:],
                            in_offset=bass.IndirectOffsetOnAxis(
                                ap=ridx_sb[:, w0 + b0:w0 + b1], axis=0))
                    if ABLATE == "nocompute":
                        continue

                    osb_c = pbo.tile([P, nwin * HU], f32, tag="osb")
                    osb3 = osb_c[:].rearrange("p (i c) -> p i c", c=HU)
                    off = 0
                    for i in range(nwin):
                        w = w0 + i
                        k = K[w]
                        if k == 0:
                            nc.gpsimd.memset(osb3[:, i, :], 0.0)
                            continue
                        lg = pbw.tile([P, k * HEADS], f16, tag="lg")
                        lg3 = lg[:].rearrange("p (j h) -> p j h", h=HEADS)
                        nc.vector.tensor_tensor(
                            out=lg3,
                            in0=hs3[:, off:off + k, S1OFF:S1OFF + HEADS],
                            in1=rg3[:, i, S2OFF:S2OFF + HEADS].unsqueeze(
                                1).broadcast_to([P, k, HEADS]),
                            op=mybir.AluOpType.add)
                        neg = pbw.tile([P, k * HEADS], f16, tag="neg")
                        nc.vector.tensor_scalar(
                            out=neg[:], in0=lg[:], scalar1=0.0,
                            scalar2=LEAKY_ALPHA, op0=mybir.AluOpType.min,
                            op1=mybir.AluOpType.mult)
                        lr = pbw.tile([P, k * HEADS], f16, tag="lr")
                        nc.vector.scalar_tensor_tensor(
                            out=lr[:], in0=lg[:], scalar=0.0, in1=neg[:],
                            op0=mybir.AluOpType.max, op1=mybir.AluOpType.add)
                        expo = pbw.tile([P, k * HEADS], f16, tag="expo")
                        nc.scalar.activation(
                            out=expo[:], in_=lr[:],
                            func=mybir.ActivationFunctionType.Exp,
                            bias=cbias[:])
                        ex3 = expo[:].rearrange("p (j h) -> p j h", h=HEADS)
                        rhs = pbw.tile([P, k * HU], f16, tag="rhs")
                        nc.vector.tensor_tensor(
                            out=rhs[:].rearrange("p (j h u) -> p j h u",
                                                 h=HEADS, u=UNITS),
                            in0=hs3[:, off:off + k, 0:HU].rearrange(
                                "p j (h u) -> p j h u", u=UNITS),
                            in1=ex3.unsqueeze(3).broadcast_to(
                                [P, k, HEADS, UNITS]),
                            op=mybir.AluOpType.mult)
                        den = pbw.tile([P, HEADS], f32, tag="den")
                        nc.vector.tensor_reduce(
                            out=den[:],
                            in_=expo[:].rearrange("p (j h) -> p h j", h=HEADS),
                            axis=mybir.AxisListType.X, op=mybir.AluOpType.add)
                        num = pbw.tile([P, HU], f32, tag="num")
                        nc.vector.tensor_reduce(
                            out=num[:],
                            in_=rhs[:].rearrange("p (j c) -> p c j", c=HU),
                            axis=mybir.AxisListType.X, op=mybir.AluOpType.add)
                        den2 = pbw.tile([P, HEADS], f32, tag="den2")
                        nc.vector.tensor_scalar_add(
                            out=den2[:], in0=den[:], scalar1=1e-30)
                        rec = pbw.tile([P, HEADS], f32, tag="rec")
                        nc.vector.reciprocal(out=rec[:], in_=den2[:])
                        nc.vector.tensor_tensor(
                            out=osb3[:, i, :].rearrange("p (h u) -> p h u",
                                                        u=UNITS),
                            in0=num[:].rearrange("p (h u) -> p h u", u=UNITS),
                            in1=rec[:].unsqueeze(2).broadcast_to(
                                [P, HEADS, UNITS]),
                            op=mybir.AluOpType.mult)
                        off += k
                    nc.sync.dma_start(
                        out=out_d[w0 * P:(w0 + nwin) * P, :].rearrange(
                            "(i p) c -> p i c", p=P),
                        in_=osb3)

    nc.compile()
    return nc


def _run(nc, per_core, n_cores):
    from concourse import bass_utils

    want_trace = bool(os.environ.get("GAT_TRACE"))
    res = bass_utils.run_bass_kernel_spmd(
        nc, per_core, core_ids=list(range(n_cores)), trace=want_trace)
    return res


def _unshard(host, results, n_cores):
    n_nodes = host["n_nodes"]
    out = np.zeros((n_nodes, HU), dtype=np.float32)
    for c in range(n_cores):
        staged = results[c]["staged"]  # [nw*128, 64]
        wn = host["win_nodes"][c]      # [nw, 128]
        valid = wn >= 0
        out[wn[valid]] = staged.reshape(wn.shape[0], P, HU)[valid]
    return out


def kernel(x, edge_index, W, att_w1, att_w2, n_cores=8, _return_results=False):
    x = np.asarray(x)
    edge_index = np.asarray(edge_index)
    W = np.asarray(W).astype(np.float32)
    att_w1 = np.asarray(att_w1).astype(np.float32)
    att_w2 = np.asarray(att_w2).astype(np.float32)

    host, per_core = _build_host_data(x, edge_index, W, att_w1, att_w2, n_cores)
    nc = _build_bass(host["plan"])
    res = _run(nc, per_core, n_cores)
    out = _unshard(host, res.results, n_cores)
    if _return_results:
        return out, res
    return out


# revision 7
# speedup vs baseline: 1.0001x; 1.0001x over previous
"""GAT (graph attention) Bass kernel for Trainium2, 8-core SPMD — v2.

Strategy: receiver-per-partition windows + chunked indirect-DMA gathers.

Host sorts active nodes by degree and packs them 128 per window (one SBUF
partition per receiver). Windows are dealt round-robin to the 8 cores so
every core runs one shared instruction stream; the per-window slot count
K[w] (edge blocks of 128 slots) is the max over the 8 cores' windows.

Device kernel, per core:
  phase A: tab[n] = [h(64) | s1(4) | s2(4) | pad] fp16 256B rows, written
           block-permuted so the stores are fully contiguous (the gather
           indices absorb the permutation); one sentinel row at npad with
           h=0, s1=-100 (=> pad slots exp to exactly 0 in fp16).
  phase B: per chunk of windows, indirect DMAs ([128, 1] i32 offsets, one
           per 128-row block — wider offset APs gather garbage on this HW)
           fetch all sender rows plus one receiver row per window.
           Compute is pure DVE/ACT per partition: logit = s1 + s2(recv),
           leaky-relu, exp(.-3.5) (softmax-shift-invariant), then weighted
           free-axis reductions. No matmuls in phase B, no collectives.

Host scatters the staged [128, 64] window outputs back to node order.
"""

import os
import sys

import numpy as np

for _p in ("/opt/trn_rl_repo", os.path.expanduser("~/.axon_site/_ro/trn_rl_repo")):
    if os.path.isdir(_p) and _p not in sys.path:
        sys.path.insert(0, _p)

P = 128
XTILE = 1024                 # phase-A node super-tile
NBLK = XTILE // P            # 8
TCOLS = 128                  # fp16 table row = 256B
HEADS = 4
UNITS = 16
HU = HEADS * UNITS           # 64
S1OFF = HU                   # cols 64:68 = s1
S2OFF = HU + HEADS           # cols 68:72 = s2
WC = HU + 2 * HEADS          # 72 written cols
LEAKY_ALPHA = 0.2
CSHIFT = 3.5                 # global exp shift (softmax-invariant)
S1_SENTINEL = -100.0         # sentinel row: exp(leaky(s1+s2)-c) == 0 in fp16
BCAP = 128                   # max edge blocks per gather chunk
WCAP = 8                     # max windows per gather chunk
GCALL = 1                    # blocks per indirect DMA call (multi-col offset
                             # APs gather garbage on HW; keep 1)
QSPLIT = 1                   # SWDGE queues for indirect calls (2 was HW-
                             # correct but no faster; desc-gen is serial)
ABLATE = "full"              # dev-only: "phaseA" | "nocompute"
REPS = 1                     # dev-only: replicate kernel body for timing


def _perm(n):
    """Node id -> permuted table row (phase-A stores become contiguous)."""
    n = np.asarray(n)
    t, r = n // XTILE, n % XTILE
    return t * XTILE + (r % P) * NBLK + (r // P)


def _build_host_data(x, edge_index, W, att_w1, att_w2, n_cores):
    n_nodes, in_feat = x.shape
    snd = edge_index[:, 0].astype(np.int64)
    rcv = edge_index[:, 1].astype(np.int64)

    ntiles = -(-n_nodes // XTILE)
    npad = ntiles * XTILE
    sent = npad  # sentinel row index

    deg = np.bincount(rcv, minlength=n_nodes)
    active = np.nonzero(deg > 0)[0]
    order_n = active[np.argsort(deg[active], kind="stable")]

    wtot = -(-len(order_n) // P)
    nw = -(-wtot // n_cores)
    wpad = nw * n_cores
    win_nodes_g = np.full((wpad, P), -1, dtype=np.int64)
    win_nodes_g.reshape(-1)[: len(order_n)] = order_n

    deg_g = np.where(win_nodes_g >= 0, deg[win_nodes_g], 0)
    k_g = deg_g.max(axis=1)
    # per-local-window block cap: max over the n_cores interleaved windows
    K = k_g.reshape(nw, n_cores).max(axis=1).astype(np.int64)

    # chunking: greedy, <= BCAP blocks and <= WCAP windows per chunk
    chunks = []  # list of (w0, nwin)
    w = 0
    while w < nw:
        w0 = w
        blocks = 0
        while w < nw and (w - w0) < WCAP and (blocks + K[w]) <= max(BCAP, K[w]):
            blocks += K[w]
            w += 1
        chunks.append((w0, w - w0))

    # node -> (core, local w, partition)
    node_c = np.full(n_nodes, -1, dtype=np.int64)
    node_w = np.zeros(n_nodes, dtype=np.int64)
    node_p = np.zeros(n_nodes, dtype=np.int64)
    gwin = np.repeat(np.arange(wpad), P).reshape(wpad, P)
    valid = win_nodes_g >= 0
    vn = win_nodes_g[valid]
    node_c[vn] = gwin[valid] % n_cores
    node_w[vn] = gwin[valid] // n_cores
    node_p[vn] = np.tile(np.arange(P), wpad).reshape(wpad, P)[valid]

    # edge -> slot k within its receiver's run
    eorder = np.argsort(rcv, kind="stable")
    rs = rcv[eorder]
    ss = snd[eorder]
    starts = np.zeros(n_nodes + 1, dtype=np.int64)
    starts[1:] = np.cumsum(deg)
    k_e = np.arange(len(rs)) - starts[rs]
    perm_ss = _perm(ss)

    base = np.zeros(nw + 1, dtype=np.int64)
    base[1:] = np.cumsum(K)
    btot = int(base[-1])  # total sender blocks per core

    xT16 = np.zeros((in_feat, npad), dtype=np.float16)
    xT16[:, :n_nodes] = np.ascontiguousarray(x.T).astype(np.float16)

    # wcat = [W | W@A1 | W@A2] fp16  [in_feat, 72]
    A12 = np.zeros((HU, 2 * HEADS), dtype=np.float32)
    for h in range(HEADS):
        A12[h * UNITS:(h + 1) * UNITS, h] = att_w1[h, 0]
        A12[h * UNITS:(h + 1) * UNITS, HEADS + h] = att_w2[h, 0]
    wcat = np.zeros((in_feat, WC), dtype=np.float32)
    wcat[:, :HU] = W
    wcat[:, HU:] = W @ A12
    wcat16 = wcat.astype(np.float16)

    zrow = np.zeros((1, TCOLS), dtype=np.float16)
    zrow[0, S1OFF:S1OFF + HEADS] = S1_SENTINEL

    per_core = []
    win_nodes_c_all = []
    for c in range(n_cores):
        wn = win_nodes_g[c::n_cores]  # [nw, 128]
        emask = node_c[rs] == c
        er = rs[emask]
        ew = node_w[er]
        ep = node_p[er]
        ek = k_e[emask]

        sidx = np.full((btot, P), sent, dtype=np.int32)  # [block, partition]
        sidx[base[ew] + ek, ep] = perm_ss[emask].astype(np.int32)
        ridx = np.where(wn >= 0, _perm(np.maximum(wn, 0)), sent).astype(np.int32)

        per_core.append({
            "xT16": xT16,
            "wcat": wcat16,
            "zrow": zrow,
            "sidx": np.ascontiguousarray(sidx.T),   # [128, btot] i32
            "ridx": np.ascontiguousarray(ridx.T),   # [128, nw] i32
        })
        win_nodes_c_all.append(wn)

    plan = {
        "npad": npad, "ntiles": ntiles, "nw": nw,
        "K": K.tolist(), "base": base.tolist(), "btot": btot,
        "chunks": chunks, "in_feat": in_feat,
    }
    host = {"plan": plan, "win_nodes": win_nodes_c_all, "n_nodes": n_nodes}
    return host, per_core


def _build_bass(plan):
    from concourse import bacc, mybir, tile
    import concourse.bass as bass

    f16 = mybir.dt.float16
    f32 = mybir.dt.float32
    i32 = mybir.dt.int32

    npad = plan["npad"]
    ntiles = plan["ntiles"]
    nw = plan["nw"]
    K = plan["K"]
    base = plan["base"]
    btot = plan["btot"]
    chunks = plan["chunks"]
    in_feat = plan["in_feat"]

    nc = bacc.Bacc("TRN2", target_bir_lowering=False, debug=False,
                   enable_asserts=False, num_devices=1,
                   num_swdge_queues=QSPLIT)
    _gq = [0]

    def _indirect(**kw):
        r = nc.gpsimd.indirect_dma_start(**kw)
        q = _gq[0] % QSPLIT
        if q:
            r.ins.queue = f"qPoolDynamic{q}"
        _gq[0] += 1
        return r

    xT_d = nc.dram_tensor("xT16", [in_feat, npad], f16, kind="ExternalInput").ap()
    wcat_d = nc.dram_tensor("wcat", [in_feat, WC], f16, kind="ExternalInput").ap()
    zrow_d = nc.dram_tensor("zrow", [1, TCOLS], f16, kind="ExternalInput").ap()
    sidx_d = nc.dram_tensor("sidx", [P, btot], i32, kind="ExternalInput").ap()
    ridx_d = nc.dram_tensor("ridx", [P, nw], i32, kind="ExternalInput").ap()

    out_d = nc.dram_tensor("staged", [nw * P, HU], f32, kind="ExternalOutput").ap()
    tab_d = nc.dram_tensor("tab", [npad + 1, TCOLS], f16, kind="Internal").ap()

    with tile.TileContext(nc) as tc:
        with tc.tile_pool(name="consts", bufs=1) as cpool:
            wcat_sb = cpool.tile([in_feat, WC], f16, tag="wcat")
            nc.sync.dma_start(out=wcat_sb[:], in_=wcat_d[:])
            sidx_sb = cpool.tile([P, btot], i32, tag="sidx")
            nc.sync.dma_start(out=sidx_sb[:], in_=sidx_d[:])
            ridx_sb = cpool.tile([P, nw], i32, tag="ridx")
            nc.sync.dma_start(out=ridx_sb[:], in_=ridx_d[:])
            zr_sb = cpool.tile([1, TCOLS], f16, tag="zrow")
            nc.sync.dma_start(out=zr_sb[:], in_=zrow_d[:])
            nc.sync.dma_start(out=tab_d[npad:npad + 1, :], in_=zr_sb[:])
            cbias = cpool.tile([P, 1], f32, tag="cbias")
            nc.gpsimd.memset(cbias[:], -CSHIFT)
            zpad = cpool.tile([P, NBLK * (TCOLS - WC)], f16, tag="zpad")
            nc.gpsimd.memset(zpad[:], 0.0)

            # ---- phase A: node table ----
            with tc.tile_pool(name="pa_x", bufs=3) as pax, \
                 tc.tile_pool(name="pa_ps", bufs=3, space="PSUM") as paps, \
                 tc.tile_pool(name="pa_hs", bufs=3) as pahs:
              for _rep in range(REPS):
                for t in range(ntiles):
                    xt = pax.tile([in_feat, XTILE], f16, tag="xt")
                    nc.sync.dma_start(
                        out=xt[:], in_=xT_d[:, t * XTILE:(t + 1) * XTILE])
                    hst = pahs.tile([P, NBLK * TCOLS], f16, tag="hst")
                    hst3 = hst[:].rearrange("p (i c) -> p i c", c=TCOLS)
                    nc.vector.tensor_copy(
                        out=hst3[:, :, WC:TCOLS],
                        in_=zpad[:].rearrange("p (i c) -> p i c",
                                              c=TCOLS - WC))
                    half = NBLK // 2
                    for g in range(2):
                        ps = paps.tile([P, half * WC], f32, tag="ps")
                        for i in range(half):
                            b = g * half + i
                            nc.tensor.matmul(
                                out=ps[:, i * WC:(i + 1) * WC],
                                lhsT=xt[:, b * P:(b + 1) * P],
                                rhs=wcat_sb[:], start=True, stop=True)
                        nc.vector.tensor_copy(
                            out=hst3[:, g * half:(g + 1) * half, 0:WC],
                            in_=ps[:].rearrange("p (i c) -> p i c", c=WC))
                    nc.sync.dma_start(
                        out=tab_d[t * XTILE:(t + 1) * XTILE, :].rearrange(
                            "(p i) c -> p i c", p=P),
                        in_=hst3)

            # ---- phase B: windows ----
            if ABLATE != "phaseA":
              with tc.tile_pool(name="pb_hs", bufs=2) as pbh, \
                   tc.tile_pool(name="pb_rg", bufs=2) as pbr, \
                   tc.tile_pool(name="pb_w", bufs=3) as pbw, \
                   tc.tile_pool(name="pb_o", bufs=2) as pbo:
                for _rep in range(REPS):
                  for (w0, nwin) in chunks:
                    nb = sum(K[w0:w0 + nwin])
                    if nb == 0:
                        continue
                    hs = pbh.tile([P, nb * TCOLS], f16, tag="hs")
                    hs3 = hs[:].rearrange("p (j c) -> p j c", c=TCOLS)
                    for b0 in range(0, nb, GCALL):
                        b1 = min(b0 + GCALL, nb)
                        _indirect(
                            out=hs3[:, b0:b1, :] if b1 - b0 > 1
                            else hs3[:, b0, :],
                            out_offset=None, in_=tab_d[:],
                            in_offset=bass.IndirectOffsetOnAxis(
                                ap=sidx_sb[:, base[w0] + b0:base[w0] + b1],
                                axis=0))
                    rg = pbr.tile([P, nwin * TCOLS], f16, tag="rg")
                    rg3 = rg[:].rearrange("p (j c) -> p j c", c=TCOLS)
                    for b0 in range(0, nwin, GCALL):
                        b1 = min(b0 + GCALL, nwin)
                        _indirect(
                            out=rg3[:, b0:b1, :] if b1 - b0 > 1
                            else rg3[:, b0, :],
                            out_offset=None, in_=tab_d[:],
                            in_offset=bass.IndirectOffsetOnAxis(
                                ap=ridx_sb[:, w0 + b0:w0 + b1], axis=0))
                    if ABLATE == "nocompute":
                        continue

                    osb_c = pbo.tile([P, nwin * HU], f32, tag="osb")
                    osb3 = osb_c[:].rearrange("p (i c) -> p i c", c=HU)
                    off = 0
                    for i in range(nwin):
                        w = w0 + i
                        k = K[w]
                        if k == 0:
                            nc.gpsimd.memset(osb3[:, i, :], 0.0)
                            continue
                        lg = pbw.tile([P, k * HEADS], f16, tag="lg")
                        lg3 = lg[:].rearrange("p (j h) -> p j h", h=HEADS)
                        nc.vector.tensor_tensor(
                            out=lg3,
                            in0=hs3[:, off:off + k, S1OFF:S1OFF + HEADS],
                            in1=rg3[:, i, S2OFF:S2OFF + HEADS].unsqueeze(
                                1).broadcast_to([P, k, HEADS]),
                            op=mybir.AluOpType.add)
                        neg = pbw.tile([P, k * HEADS], f16, tag="neg")
                        nc.vector.tensor_scalar(
                            out=neg[:], in0=lg[:], scalar1=0.0,
                            scalar2=LEAKY_ALPHA, op0=mybir.AluOpType.min,
                            op1=mybir.AluOpType.mult)
                        lr = pbw.tile([P, k * HEADS], f16, tag="lr")
                        nc.vector.scalar_tensor_tensor(
                            out=lr[:], in0=lg[:], scalar=0.0, in1=neg[:],
                            op0=mybir.AluOpType.max, op1=mybir.AluOpType.add)
                        expo = pbw.tile([P, k * HEADS], f16, tag="expo")
                        nc.scalar.activation(
                            out=expo[:], in_=lr[:],
                            func=mybir.ActivationFunctionType.Exp,
                            bias=cbias[:])
                        ex3 = expo[:].rearrange("p (j h) -> p j h", h=HEADS)
                        rhs = pbw.tile([P, k * HU], f16, tag="rhs")
                        nc.vector.tensor_tensor(
                            out=rhs[:].rearrange("p (j h u) -> p j h u",
                                                 h=HEADS, u=UNITS),
                            in0=hs3[:, off:off + k, 0:HU].rearrange(
                                "p j (h u) -> p j h u", u=UNITS),
                            in1=ex3.unsqueeze(3).broadcast_to(
                                [P, k, HEADS, UNITS]),
                            op=mybir.AluOpType.mult)
                        den = pbw.tile([P, HEADS], f32, tag="den")
                        nc.vector.tensor_reduce(
                            out=den[:],
                            in_=expo[:].rearrange("p (j h) -> p h j", h=HEADS),
                            axis=mybir.AxisListType.X, op=mybir.AluOpType.add)
                        num = pbw.tile([P, HU], f32, tag="num")
                        nc.vector.tensor_reduce(
                            out=num[:],
                            in_=rhs[:].rearrange("p (j c) -> p c j", c=HU),
                            axis=mybir.AxisListType.X, op=mybir.AluOpType.add)
                        den2 = pbw.tile([P, HEADS], f32, tag="den2")
                        nc.vector.tensor_scalar_add(
                            out=den2[:], in0=den[:], scalar1=1e-30)
                        rec = pbw.tile([P, HEADS], f32, tag="rec")
                        nc.vector.reciprocal(out=rec[:], in_=den2[:])
                        nc.vector.tensor_tensor(
                            out=osb3[:, i, :].rearrange("p (h u) -> p h u",
                                                        u=UNITS),
                            in0=num[:].rearrange("p (h u) -> p h u", u=UNITS),
                            in1=rec[:].unsqueeze(2).broadcast_to(
                                [P, HEADS, UNITS]),
                            op=mybir.AluOpType.mult)
                        off += k
                    nc.sync.dma_start(
                        out=out_d[w0 * P:(w0 + nwin) * P, :].rearrange(
                            "(i p) c -> p i c", p=P),
                        in_=osb3)

    nc.compile()
    return nc


def _run(nc, per_core, n_cores):
    from concourse import bass_utils

    want_trace = bool(os.environ.get("GAT_TRACE"))
    res = bass_utils.run_bass_kernel_spmd(
        nc, per_core, core_ids=list(range(n_cores)), trace=want_trace)
    return res


def _unshard(host, results, n_cores):
    n_nodes = host["n_nodes"]
    out = np.zeros((n_nodes, HU), dtype=np.float32)
    for c in range(n_cores):
        staged = results[c]["staged"]  # [nw*128, 64]
        wn = host["win_nodes"][c]      # [nw, 128]
        valid = wn >= 0
        out[wn[valid]] = staged.reshape(wn.shape[0], P, HU)[valid]
    return out


def kernel(x, edge_index, W, att_w1, att_w2, n_cores=8, _return_results=False):
    x = np.asarray(x)
    edge_index = np.asarray(edge_index)
    W = np.asarray(W).astype(np.float32)
    att_w1 = np.asarray(att_w1).astype(np.float32)
    att_w2 = np.asarray(att_w2).astype(np.float32)

    host, per_core = _build_host_data(x, edge_index, W, att_w1, att_w2, n_cores)
    nc = _build_bass(host["plan"])
    res = _run(nc, per_core, n_cores)
    out = _unshard(host, res.results, n_cores)
    if _return_results:
        return out, res
    return out


# revision 9
# speedup vs baseline: 1.1460x; 1.1459x over previous
"""GAT (graph attention) Bass kernel for Trainium2, 8-core SPMD — v2.

Strategy: receiver-per-partition windows + chunked indirect-DMA gathers.

Host sorts active nodes by degree and packs them 128 per window (one SBUF
partition per receiver). Windows are dealt round-robin to the 8 cores so
every core runs one shared instruction stream; the per-window slot count
K[w] (edge blocks of 128 slots) is the max over the 8 cores' windows.

Device kernel, per core:
  phase A: tab[n] = [h(64) | s1(4) | s2(4) | pad] fp16 256B rows, written
           block-permuted so the stores are fully contiguous (the gather
           indices absorb the permutation); one sentinel row at npad with
           h=0, s1=-100 (=> pad slots exp to exactly 0 in fp16).
  phase B: per chunk of windows, indirect DMAs ([128, 1] i32 offsets, one
           per 128-row block — wider offset APs gather garbage on this HW)
           fetch all sender rows plus one receiver row per window.
           Compute is pure DVE/ACT per partition: logit = s1 + s2(recv),
           leaky-relu, exp(.-3.5) (softmax-shift-invariant), then weighted
           free-axis reductions. No matmuls in phase B, no collectives.

Host scatters the staged [128, 64] window outputs back to node order.
"""

import os
import sys

import numpy as np

for _p in ("/opt/trn_rl_repo", os.path.expanduser("~/.axon_site/_ro/trn_rl_repo")):
    if os.path.isdir(_p) and _p not in sys.path:
        sys.path.insert(0, _p)

P = 128
XTILE = 1024                 # phase-A node super-tile
NBLK = XTILE // P            # 8
TCOLS = 128                  # fp16 table row = 256B
HEADS = 4
UNITS = 16
HU = HEADS * UNITS           # 64
S1OFF = HU                   # cols 64:68 = s1
S2OFF = HU + HEADS           # cols 68:72 = s2
WC = HU + 2 * HEADS          # 72 written cols
LEAKY_ALPHA = 0.2
CSHIFT = 3.5                 # global exp shift (softmax-invariant)
S1_SENTINEL = -100.0         # sentinel row: exp(leaky(s1+s2)-c) == 0 in fp16
BCAP = 128                   # max edge blocks per gather chunk
WCAP = 8                     # max windows per gather chunk
GCALL = 1                    # blocks per indirect DMA call (multi-col offset
                             # APs gather garbage on HW; keep 1)
QSPLIT = 1                   # SWDGE queues for indirect calls (2 was HW-
                             # correct but no faster; desc-gen is serial)
ABLATE = "full"              # dev-only: "phaseA" | "nocompute"
REPS = 1                     # dev-only: replicate kernel body for timing


def _perm(n):
    """Node id -> permuted table row (phase-A stores become contiguous)."""
    n = np.asarray(n)
    t, r = n // XTILE, n % XTILE
    return t * XTILE + (r % P) * NBLK + (r // P)


def _build_host_data(x, edge_index, W, att_w1, att_w2, n_cores):
    n_nodes, in_feat = x.shape
    snd = edge_index[:, 0].astype(np.int64)
    rcv = edge_index[:, 1].astype(np.int64)

    ntiles = -(-n_nodes // XTILE)
    npad = ntiles * XTILE
    sent = npad  # sentinel row index

    deg = np.bincount(rcv, minlength=n_nodes)
    active = np.nonzero(deg > 0)[0]
    order_n = active[np.argsort(deg[active], kind="stable")]

    wtot = -(-len(order_n) // P)
    nw = -(-wtot // n_cores)
    wpad = nw * n_cores
    win_nodes_g = np.full((wpad, P), -1, dtype=np.int64)
    win_nodes_g.reshape(-1)[: len(order_n)] = order_n

    deg_g = np.where(win_nodes_g >= 0, deg[win_nodes_g], 0)
    k_g = deg_g.max(axis=1)
    # per-local-window block cap: max over the n_cores interleaved windows
    K = k_g.reshape(nw, n_cores).max(axis=1).astype(np.int64)

    # chunking: greedy, <= BCAP blocks and <= WCAP windows per chunk
    chunks = []  # list of (w0, nwin)
    w = 0
    while w < nw:
        w0 = w
        blocks = 0
        while w < nw and (w - w0) < WCAP and (blocks + K[w]) <= max(BCAP, K[w]):
            blocks += K[w]
            w += 1
        chunks.append((w0, w - w0))

    # node -> (core, local w, partition)
    node_c = np.full(n_nodes, -1, dtype=np.int64)
    node_w = np.zeros(n_nodes, dtype=np.int64)
    node_p = np.zeros(n_nodes, dtype=np.int64)
    gwin = np.repeat(np.arange(wpad), P).reshape(wpad, P)
    valid = win_nodes_g >= 0
    vn = win_nodes_g[valid]
    node_c[vn] = gwin[valid] % n_cores
    node_w[vn] = gwin[valid] // n_cores
    node_p[vn] = np.tile(np.arange(P), wpad).reshape(wpad, P)[valid]

    # edge -> slot k within its receiver's run
    eorder = np.argsort(rcv, kind="stable")
    rs = rcv[eorder]
    ss = snd[eorder]
    starts = np.zeros(n_nodes + 1, dtype=np.int64)
    starts[1:] = np.cumsum(deg)
    k_e = np.arange(len(rs)) - starts[rs]
    perm_ss = _perm(ss)

    base = np.zeros(nw + 1, dtype=np.int64)
    base[1:] = np.cumsum(K)
    btot = int(base[-1])  # total sender blocks per core

    xT16 = np.zeros((in_feat, npad), dtype=np.float16)
    xT16[:, :n_nodes] = np.ascontiguousarray(x.T).astype(np.float16)

    # wcat = [W | W@A1 | W@A2] fp16  [in_feat, 72]
    A12 = np.zeros((HU, 2 * HEADS), dtype=np.float32)
    for h in range(HEADS):
        A12[h * UNITS:(h + 1) * UNITS, h] = att_w1[h, 0]
        A12[h * UNITS:(h + 1) * UNITS, HEADS + h] = att_w2[h, 0]
    wcat = np.zeros((in_feat, WC), dtype=np.float32)
    wcat[:, :HU] = W
    wcat[:, HU:] = W @ A12
    wcat16 = wcat.astype(np.float16)

    zrow = np.zeros((1, TCOLS), dtype=np.float16)
    zrow[0, S1OFF:S1OFF + HEADS] = S1_SENTINEL

    per_core = []
    win_nodes_c_all = []
    for c in range(n_cores):
        wn = win_nodes_g[c::n_cores]  # [nw, 128]
        emask = node_c[rs] == c
        er = rs[emask]
        ew = node_w[er]
        ep = node_p[er]
        ek = k_e[emask]

        sidx = np.full((btot, P), sent, dtype=np.int32)  # [block, partition]
        sidx[base[ew] + ek, ep] = perm_ss[emask].astype(np.int32)
        ridx = np.where(wn >= 0, _perm(np.maximum(wn, 0)), sent).astype(np.int32)

        per_core.append({
            "xT16": xT16,
            "wcat": wcat16,
            "zrow": zrow,
            "sidx": np.ascontiguousarray(sidx.T),   # [128, btot] i32
            "ridx": np.ascontiguousarray(ridx.T),   # [128, nw] i32
        })
        win_nodes_c_all.append(wn)

    plan = {
        "npad": npad, "ntiles": ntiles, "nw": nw,
        "K": K.tolist(), "base": base.tolist(), "btot": btot,
        "chunks": chunks, "in_feat": in_feat,
    }
    host = {"plan": plan, "win_nodes": win_nodes_c_all, "n_nodes": n_nodes}
    return host, per_core


def _build_bass(plan):
    from concourse import bacc, mybir, tile
    import concourse.bass as bass

    f16 = mybir.dt.float16
    f32 = mybir.dt.float32
    i32 = mybir.dt.int32

    npad = plan["npad"]
    ntiles = plan["ntiles"]
    nw = plan["nw"]
    K = plan["K"]
    base = plan["base"]
    btot = plan["btot"]
    chunks = plan["chunks"]
    in_feat = plan["in_feat"]

    nc = bacc.Bacc("TRN2", target_bir_lowering=False, debug=False,
                   enable_asserts=False, num_devices=1,
                   num_swdge_queues=QSPLIT)
    _gq = [0]

    def _indirect(**kw):
        r = nc.gpsimd.indirect_dma_start(**kw)
        q = _gq[0] % QSPLIT
        if q:
            r.ins.queue = f"qPoolDynamic{q}"
        _gq[0] += 1
        return r

    xT_d = nc.dram_tensor("xT16", [in_feat, npad], f16, kind="ExternalInput").ap()
    wcat_d = nc.dram_tensor("wcat", [in_feat, WC], f16, kind="ExternalInput").ap()
    zrow_d = nc.dram_tensor("zrow", [1, TCOLS], f16, kind="ExternalInput").ap()
    sidx_d = nc.dram_tensor("sidx", [P, btot], i32, kind="ExternalInput").ap()
    ridx_d = nc.dram_tensor("ridx", [P, nw], i32, kind="ExternalInput").ap()

    out_d = nc.dram_tensor("staged", [nw * P, HU], f32, kind="ExternalOutput").ap()
    tab_d = nc.dram_tensor("tab", [npad + 1, TCOLS], f16, kind="Internal").ap()

    with tile.TileContext(nc) as tc:
        with tc.tile_pool(name="consts", bufs=1) as cpool:
            wcat_sb = cpool.tile([in_feat, WC], f16, tag="wcat")
            nc.sync.dma_start(out=wcat_sb[:], in_=wcat_d[:])
            sidx_sb = cpool.tile([P, btot], i32, tag="sidx")
            nc.sync.dma_start(out=sidx_sb[:], in_=sidx_d[:])
            ridx_sb = cpool.tile([P, nw], i32, tag="ridx")
            nc.sync.dma_start(out=ridx_sb[:], in_=ridx_d[:])
            zr_sb = cpool.tile([1, TCOLS], f16, tag="zrow")
            nc.sync.dma_start(out=zr_sb[:], in_=zrow_d[:])
            nc.sync.dma_start(out=tab_d[npad:npad + 1, :], in_=zr_sb[:])
            cbias = cpool.tile([P, 1], f32, tag="cbias")
            nc.gpsimd.memset(cbias[:], -CSHIFT)
            zpad = cpool.tile([P, NBLK * (TCOLS - WC)], f16, tag="zpad")
            nc.gpsimd.memset(zpad[:], 0.0)

            # ---- phase A: node table ----
            with tc.tile_pool(name="pa_x", bufs=3) as pax, \
                 tc.tile_pool(name="pa_ps", bufs=3, space="PSUM") as paps, \
                 tc.tile_pool(name="pa_hs", bufs=3) as pahs:
              for _rep in range(REPS):
                for t in range(ntiles):
                    xt = pax.tile([in_feat, XTILE], f16, tag="xt")
                    nc.sync.dma_start(
                        out=xt[:], in_=xT_d[:, t * XTILE:(t + 1) * XTILE])
                    hst = pahs.tile([P, NBLK * TCOLS], f16, tag="hst")
                    hst3 = hst[:].rearrange("p (i c) -> p i c", c=TCOLS)
                    nc.vector.tensor_copy(
                        out=hst3[:, :, WC:TCOLS],
                        in_=zpad[:].rearrange("p (i c) -> p i c",
                                              c=TCOLS - WC))
                    half = NBLK // 2
                    for g in range(2):
                        ps = paps.tile([P, half * WC], f32, tag="ps")
                        for i in range(half):
                            b = g * half + i
                            nc.tensor.matmul(
                                out=ps[:, i * WC:(i + 1) * WC],
                                lhsT=xt[:, b * P:(b + 1) * P],
                                rhs=wcat_sb[:], start=True, stop=True)
                        nc.vector.tensor_copy(
                            out=hst3[:, g * half:(g + 1) * half, 0:WC],
                            in_=ps[:].rearrange("p (i c) -> p i c", c=WC))
                    nc.sync.dma_start(
                        out=tab_d[t * XTILE:(t + 1) * XTILE, :].rearrange(
                            "(p i) c -> p i c", p=P),
                        in_=hst3)

            # ---- phase B: windows ----
            if ABLATE != "phaseA":
              with tc.tile_pool(name="pb_hs", bufs=2) as pbh, \
                   tc.tile_pool(name="pb_rg", bufs=2) as pbr, \
                   tc.tile_pool(name="pb_w", bufs=3) as pbw, \
                   tc.tile_pool(name="pb_o", bufs=2) as pbo:
                for _rep in range(REPS):
                  for (w0, nwin) in chunks:
                    nb = sum(K[w0:w0 + nwin])
                    if nb == 0:
                        continue
                    hs = pbh.tile([P, nb * TCOLS], f16, tag="hs")
                    hs3 = hs[:].rearrange("p (j c) -> p j c", c=TCOLS)
                    for b0 in range(0, nb, GCALL):
                        b1 = min(b0 + GCALL, nb)
                        _indirect(
                            out=hs3[:, b0:b1, :] if b1 - b0 > 1
                            else hs3[:, b0, :],
                            out_offset=None, in_=tab_d[:],
                            in_offset=bass.IndirectOffsetOnAxis(
                                ap=sidx_sb[:, base[w0] + b0:base[w0] + b1],
                                axis=0))
                    rg = pbr.tile([P, nwin * TCOLS], f16, tag="rg")
                    rg3 = rg[:].rearrange("p (j c) -> p j c", c=TCOLS)
                    for b0 in range(0, nwin, GCALL):
                        b1 = min(b0 + GCALL, nwin)
                        _indirect(
                            out=rg3[:, b0:b1, :] if b1 - b0 > 1
                            else rg3[:, b0, :],
                            out_offset=None, in_=tab_d[:],
                            in_offset=bass.IndirectOffsetOnAxis(
                                ap=ridx_sb[:, w0 + b0:w0 + b1], axis=0))
                    if ABLATE == "nocompute":
                        continue

                    osb_c = pbo.tile([P, nwin * HU], f32, tag="osb")
                    osb3 = osb_c[:].rearrange("p (i c) -> p i c", c=HU)
                    off = 0
                    for i in range(nwin):
                        w = w0 + i
                        k = K[w]
                        if k == 0:
                            nc.gpsimd.memset(osb3[:, i, :], 0.0)
                            continue
                        lg = pbw.tile([P, k * HEADS], f16, tag="lg")
                        lg3 = lg[:].rearrange("p (j h) -> p j h", h=HEADS)
                        nc.vector.tensor_tensor(
                            out=lg3,
                            in0=hs3[:, off:off + k, S1OFF:S1OFF + HEADS],
                            in1=rg3[:, i, S2OFF:S2OFF + HEADS].unsqueeze(
                                1).broadcast_to([P, k, HEADS]),
                            op=mybir.AluOpType.add)
                        neg = pbw.tile([P, k * HEADS], f16, tag="neg")
                        nc.vector.tensor_scalar(
                            out=neg[:], in0=lg[:], scalar1=0.0,
                            scalar2=LEAKY_ALPHA, op0=mybir.AluOpType.min,
                            op1=mybir.AluOpType.mult)
                        lr = pbw.tile([P, k * HEADS], f16, tag="lr")
                        nc.vector.scalar_tensor_tensor(
                            out=lr[:], in0=lg[:], scalar=0.0, in1=neg[:],
                            op0=mybir.AluOpType.max, op1=mybir.AluOpType.add)
                        expo = pbw.tile([P, k * HEADS], f16, tag="expo")
                        nc.scalar.activation(
                            out=expo[:], in_=lr[:],
                            func=mybir.ActivationFunctionType.Exp,
                            bias=cbias[:])
                        ex3 = expo[:].rearrange("p (j h) -> p j h", h=HEADS)
                        rhs = pbw.tile([P, k * HU], f16, tag="rhs")
                        nc.vector.tensor_tensor(
                            out=rhs[:].rearrange("p (j h u) -> p j h u",
                                                 h=HEADS, u=UNITS),
                            in0=hs3[:, off:off + k, 0:HU].rearrange(
                                "p j (h u) -> p j h u", u=UNITS),
                            in1=ex3.unsqueeze(3).broadcast_to(
                                [P, k, HEADS, UNITS]),
                            op=mybir.AluOpType.mult)
                        den = pbw.tile([P, HEADS], f32, tag="den")
                        nc.vector.tensor_reduce(
                            out=den[:],
                            in_=expo[:].rearrange("p (j h) -> p h j", h=HEADS),
                            axis=mybir.AxisListType.X, op=mybir.AluOpType.add)
                        num = pbw.tile([P, HU], f32, tag="num")
                        nc.vector.tensor_reduce(
                            out=num[:],
                            in_=rhs[:].rearrange("p (j c) -> p c j", c=HU),
                            axis=mybir.AxisListType.X, op=mybir.AluOpType.add)
                        den2 = pbw.tile([P, HEADS], f32, tag="den2")
                        nc.vector.tensor_scalar_add(
                            out=den2[:], in0=den[:], scalar1=1e-30)
                        rec = pbw.tile([P, HEADS], f32, tag="rec")
                        nc.vector.reciprocal(out=rec[:], in_=den2[:])
                        nc.vector.tensor_tensor(
                            out=osb3[:, i, :].rearrange("p (h u) -> p h u",
                                                        u=UNITS),
                            in0=num[:].rearrange("p (h u) -> p h u", u=UNITS),
                            in1=rec[:].unsqueeze(2).broadcast_to(
                                [P, HEADS, UNITS]),
                            op=mybir.AluOpType.mult)
                        off += k
                    nc.sync.dma_start(
                        out=out_d[w0 * P:(w0 + nwin) * P, :].rearrange(
                            "(i p) c -> p i c", p=P),
                        in_=osb3)

    nc.compile()
    return nc


def _run(nc, per_core, n_cores):
    from concourse import bass_utils

    want_trace = bool(os.environ.get("GAT_TRACE"))
    res = bass_utils.run_bass_kernel_spmd(
        nc, per_core, core_ids=list(range(n_cores)), trace=want_trace)
    return res


def _unshard(host, results, n_cores):
    n_nodes = host["n_nodes"]
    out = np.zeros((n_nodes, HU), dtype=np.float32)
    for c in range(n_cores):
        staged = results[c]["staged"]  # [nw*128, 64]
        wn = host["win_nodes"][c]      # [nw, 128]
        valid = wn >= 0
        out[wn[valid]] = staged.reshape(wn.shape[0], P, HU)[valid]
    return out


def kernel(x, edge_index, W, att_w1, att_w2, n_cores=8, _return_results=False):
    x = np.asarray(x)
    edge_index = np.asarray(edge_index)
    W = np.asarray(W).astype(np.float32)
    att_w1 = np.asarray(att_w1).astype(np.float32)
    att_w2 = np.asarray(att_w2).astype(np.float32)

    host, per_core = _build_host_data(x, edge_index, W, att_w1, att_w2, n_cores)
    nc = _build_bass(host["plan"])
    res = _run(nc, per_core, n_cores)
    out = _unshard(host, res.results, n_cores)
    if _return_results:
        return out, res
    return out
